# revision 1
# baseline (speedup 1.0000x reference)
"""Trainium2 Bass kernel for a causal multi-head attention block.

Reference computation (B=4, L=2048, D=1024, H=16, dk=64):
    h = LayerNorm(x); qkv = h @ W_in.T; q,k = rope(q),rope(k)
    o = causal_softmax(q k^T / 8) v;  out = o @ W_o.T

Sharding: Megatron-style tensor parallel over heads across 8 cores
(2 heads/core). x replicated (cast to bf16 host-side); W_in
column-sharded; W_o row-sharded; host sums the 8 bf16 partial outputs
in f32.

Device kernel layout: scores are computed transposed (S^T[k, q], keys
on partitions) with exp fused off the PSUM scores; A@V accumulates per
(q-tile, k-tile) with a ones-column in V producing softmax row-sums in
the same PSUM tile; o is normalized with a per-partition reciprocal,
transposed once, and fed to the W_o projection. x tiles are DMA'd one
chunk ahead, and the rope (x1,x2) pair swap is one DVE stream_shuffle
(rows are host-permuted so the swap is a within-quadrant 16-block
exchange). Attention for batch b-1 is software-pipelined between
batch b's LN/QKV chunks.
"""
import numpy as np
import ml_dtypes

import concourse.bass as bass
import concourse.bacc as bacc
import concourse.tile as tile
from concourse import mybir
from concourse.masks import make_identity

f32 = mybir.dt.float32
bf16 = mybir.dt.bfloat16
BF = ml_dtypes.bfloat16
AF = mybir.ActivationFunctionType
OP = mybir.AluOpType

D_MODEL = 1024
HEADS = 16
D_K = 64
N_CORES = 8
HPC = HEADS // N_CORES
ROPE_BASE = 10000.0
EPS = 1e-8
DC = D_MODEL // 128


def build_nc(B, L, variant=""):
    nc = bacc.Bacc("TRN2", target_bir_lowering=False)
    G = B * L // 512
    CPB = L // 512
    KT = L // 128
    QC = L // 512

    x_d = nc.dram_tensor("x", [B, L, D_MODEL], bf16, kind="ExternalInput")
    wqk_d = nc.dram_tensor("wqkT", [D_MODEL, 256], bf16, kind="ExternalInput")
    wv_d = nc.dram_tensor("wvT", [D_MODEL, 128], bf16, kind="ExternalInput")
    wo_d = nc.dram_tensor("woT", [128, D_MODEL], bf16, kind="ExternalInput")
    cos_d = nc.dram_tensor("cosT", [128, L], bf16, kind="ExternalInput")
    sins_d = nc.dram_tensor("sinsT", [128, L], bf16, kind="ExternalInput")
    out_d = nc.dram_tensor("out", [B, L, D_MODEL], bf16, kind="ExternalOutput")

    from contextlib import ExitStack
    with tile.TileContext(nc) as tc, ExitStack() as ctx:
        csts = ctx.enter_context(tc.tile_pool(name="csts", bufs=1))
        sb_x = ctx.enter_context(tc.tile_pool(name="sb_x", bufs=8))
        sb_h = ctx.enter_context(tc.tile_pool(name="sb_h", bufs=6))
        sb_hT = ctx.enter_context(tc.tile_pool(name="sb_hT", bufs=2))
        sb_st = ctx.enter_context(tc.tile_pool(name="sb_st", bufs=6))
        sb_m = ctx.enter_context(tc.tile_pool(name="sb_m", bufs=4))
        sb_at = ctx.enter_context(tc.tile_pool(name="sb_at", bufs=3))
        sb_o = ctx.enter_context(tc.tile_pool(name="sb_o", bufs=3))
        sb_out = ctx.enter_context(tc.tile_pool(name="sb_out", bufs=4))
        ps_a = ctx.enter_context(tc.tile_pool(name="ps_a", bufs=2, space="PSUM"))
        ps_st = ctx.enter_context(tc.tile_pool(name="ps_st", bufs=2, space="PSUM"))
        ps_tr = ctx.enter_context(tc.tile_pool(name="ps_tr", bufs=2, space="PSUM"))

        # fetch chunk 0's x before the constant loads so LN stats can start
        # during the weight/table DMAs
        x_first = []
        for tt in range(4):
            xt = sb_x.tile([128, D_MODEL], bf16, tag="x", name=f"x0_{tt}")
            nc.sync.dma_start(out=xt, in_=x_d[0, tt * 128:(tt + 1) * 128, :])
            x_first.append(xt)

        wqk_sb = csts.tile([128, DC, 256], bf16)
        nc.sync.dma_start(out=wqk_sb, in_=wqk_d.rearrange("(c p) n -> p c n", p=128))
        wv_sb = csts.tile([128, DC, 128], bf16)
        nc.sync.dma_start(out=wv_sb, in_=wv_d.rearrange("(c p) n -> p c n", p=128))
        wo_sb = csts.tile([128, D_MODEL], bf16)
        nc.sync.dma_start(out=wo_sb, in_=wo_d[:, :])
        cos_sb = csts.tile([128, L], bf16)
        nc.sync.dma_start(out=cos_sb, in_=cos_d[:, :])
        sins_sb = csts.tile([128, L], bf16)
        nc.sync.dma_start(out=sins_sb, in_=sins_d[:, :])
        ident = csts.tile([128, 128], bf16)
        make_identity(nc, ident)
        mask = csts.tile([128, 128], bf16)
        nc.gpsimd.memset(mask, 1.0)
        nc.gpsimd.affine_select(out=mask, in_=mask, compare_op=OP.is_ge,
                                fill=0.0, base=0, pattern=[[1, 128]],
                                channel_multiplier=-1)

        QT = csts.tile([128, B, L], bf16)
        KTb = csts.tile([128, B, L], bf16)
        VT = csts.tile([128, B, KT, 130], bf16)
        nc.gpsimd.memset(VT[:, :, :, 64:65], 1.0)
        nc.gpsimd.memset(VT[:, :, :, 129:130], 1.0)

        SWAP16 = list(range(16, 32)) + list(range(16))

        def rope_store(src_ps, dst_ap, l0):
            s = sb_m.tile([128, 512], bf16, tag="qs")
            nc.any.tensor_copy(out=s, in_=src_ps)
            m1 = sb_m.tile([128, 512], bf16, tag="m1")
            nc.vector.tensor_tensor(out=m1, in0=s, in1=cos_sb[:, l0:l0 + 512],
                                    op=OP.mult)
            ssw = sb_m.tile([128, 512], bf16, tag="ssw")
            nc.vector.stream_shuffle(ssw, s, SWAP16)
            m2 = sb_m.tile([128, 512], bf16, tag="m2")
            nc.vector.tensor_tensor(out=m2, in0=ssw,
                                    in1=sins_sb[:, l0:l0 + 512], op=OP.mult)
            nc.vector.tensor_tensor(out=dst_ap, in0=m1, in1=m2, op=OP.add)

        def attention_block(b, qc):
            po_T = ps_tr.tile([128, 512], bf16, tag="ptr")
            nkt = 4 * qc + 4
            # Scores for BOTH heads interleaved: head a sits on PE array rows
            # 0-63 and head b on rows 64-127 (tile_position is inferred from
            # the lhsT base partition), so adjacent matmuls overlap in the
            # array. Diagonal k-tiles compute/exp only their valid query
            # suffix -- the per-qt AV loop reads At[:, 4qc+j, qt*128:] only
            # for qt >= j, so the invalid prefix is never consumed.
            Ats = [sb_at.tile([128, KT, 512], bf16, tag="at", name=f"at{h}")
                   for h in range(HPC)]
            for g2 in range(0, nkt, 2):
                psts = [ps_st.tile([128, 2, 512], f32, tag="pst", name=f"pst{h}")
                        for h in range(HPC)]
                for i in range(2):
                    kt = g2 + i
                    c0 = max(kt - 4 * qc, 0) * 128
                    for hh in range(HPC):
                        r0 = hh * 64
                        nc.tensor.matmul(
                            psts[hh][:, i, 0:512 - c0],
                            lhsT=KTb[r0:r0 + 64, b, kt * 128:(kt + 1) * 128],
                            rhs=QT[r0:r0 + 64, b, qc * 512 + c0:(qc + 1) * 512],
                            start=True, stop=True)
                if g2 + 1 < 4 * qc:        # both tiles whole: exp the pair
                    for hh in range(HPC):
                        nc.scalar.activation(Ats[hh][:, g2:g2 + 2, :], psts[hh],
                                             AF.Exp, scale=0.125)
                else:                      # diagonal tiles: exp valid suffix
                    for i in range(2):
                        kt = g2 + i
                        c0 = max(kt - 4 * qc, 0) * 128
                        for hh in range(HPC):
                            nc.scalar.activation(Ats[hh][:, kt, c0:512],
                                                 psts[hh][:, i, 0:512 - c0],
                                                 AF.Exp, scale=0.125)
            mask_eng = nc.vector if "mdve" in variant else nc.gpsimd
            for j in range(4):
                for hh in range(HPC):
                    blk = Ats[hh][:, 4 * qc + j, j * 128:(j + 1) * 128]
                    mask_eng.tensor_tensor(out=blk, in0=blk, in1=mask,
                                           op=OP.mult)
            for hh in range(HPC):
                r0 = hh * 64
                At = Ats[hh]
                po = ps_a.tile([128, 4, 65], f32, tag="psa")
                for qt in range(4):
                    lkt = 4 * qc + qt
                    for kt in range(lkt + 1):
                        nc.tensor.matmul(
                            po[:, qt, :],
                            lhsT=At[:, kt, qt * 128:(qt + 1) * 128],
                            rhs=VT[:, b, kt, hh * 65:hh * 65 + 65],
                            start=(kt == 0), stop=(kt == lkt))
                rec = sb_st.tile([128, 4, 1], f32, tag="rec")
                nc.vector.reciprocal(out=rec, in_=po[:, :, 64:65])
                o_sb = sb_o.tile([128, 4, 64], bf16, tag="osb")
                for qt in range(4):
                    nc.any.tensor_scalar(out=o_sb[:, qt, :],
                                         in0=po[:, qt, 0:64],
                                         scalar1=rec[:, qt, :], scalar2=None,
                                         op0=OP.mult)
                for qt in range(4):
                    nc.tensor.transpose(po_T[r0:r0 + 64, qt * 128:(qt + 1) * 128],
                                        o_sb[:, qt, :], ident)
            oT = sb_o.tile([128, 4, 128], bf16, tag="oT")
            nc.any.tensor_copy(out=oT, in_=po_T)
            for qt in range(4):
                pO1 = ps_a.tile([128, 512], f32, tag="psa")
                pO2 = ps_a.tile([128, 512], f32, tag="psa")
                nc.tensor.matmul(pO1, lhsT=oT[:, qt, :],
                                 rhs=wo_sb[:, 0:512], start=True, stop=True)
                nc.tensor.matmul(pO2, lhsT=oT[:, qt, :],
                                 rhs=wo_sb[:, 512:1024], start=True, stop=True)
                osb = sb_out.tile([128, D_MODEL], bf16, tag="outsb")
                nc.any.tensor_copy(out=osb[:, 0:512], in_=pO1)
                nc.any.tensor_copy(out=osb[:, 512:1024], in_=pO2)
                lq = qc * 512 + qt * 128
                nc.sync.dma_start(out=out_d[b, lq:lq + 128, :], in_=osb)

        def fetch_x(g):
            b, cq = divmod(g, CPB)
            l0 = cq * 512
            tiles = []
            for tt in range(4):
                xt = sb_x.tile([128, D_MODEL], bf16, tag="x")
                nc.sync.dma_start(out=xt, in_=x_d[b, l0 + tt * 128:l0 + (tt + 1) * 128, :])
                tiles.append(xt)
            return tiles

        x_pend = x_first
        for g in range(G):
            b, cq = divmod(g, CPB)
            l0 = cq * 512

            x_tiles = x_pend
            if g + 1 < G:
                x_pend = fetch_x(g + 1)
            mv = sb_st.tile([128, 4, 2], f32, tag="mv")
            for tt in range(4):
                st = sb_st.tile([128, 2, 6], f32, tag="stats")
                nc.vector.bn_stats(out=st[:, 0, :], in_=x_tiles[tt][:, 0:512])
                nc.vector.bn_stats(out=st[:, 1, :], in_=x_tiles[tt][:, 512:1024])
                nc.vector.bn_aggr(out=mv[:, tt, :], in_=st)

            i32 = mybir.dt.int32
            ve = sb_st.tile([128, 4, 1], f32, tag="ve")
            nc.vector.tensor_scalar(out=ve, in0=mv[:, :, 1:2], scalar1=EPS,
                                    scalar2=None, op0=OP.add)
            rsig = sb_st.tile([128, 4, 1], f32, tag="rsig")
            nc.vector.tensor_scalar(out=rsig.bitcast(i32), in0=ve.bitcast(i32),
                                    scalar1=1, scalar2=None,
                                    op0=OP.logical_shift_right)
            nc.vector.tensor_scalar(out=rsig.bitcast(i32), in0=rsig.bitcast(i32),
                                    scalar1=-1, scalar2=0x5f3759df,
                                    op0=OP.mult, op1=OP.add)
            nt = sb_st.tile([128, 4, 1], f32, tag="nt")
            for _ in range(2):
                nc.vector.tensor_tensor(out=nt, in0=rsig, in1=rsig, op=OP.mult)
                nc.vector.tensor_tensor(out=nt, in0=nt, in1=ve, op=OP.mult)
                nc.vector.tensor_scalar(out=nt, in0=nt, scalar1=-0.5, scalar2=1.5,
                                        op0=OP.mult, op1=OP.add)
                nc.vector.tensor_tensor(out=rsig, in0=rsig, in1=nt, op=OP.mult)
            mrs = sb_st.tile([128, 4, 1], f32, tag="mrs")
            nc.vector.tensor_tensor(out=mrs, in0=mv[:, :, 0:1], in1=rsig, op=OP.mult)

            h_tiles = []
            for tt in range(4):
                ht = sb_h.tile([128, D_MODEL], bf16, tag="h")
                nc.any.tensor_scalar(out=ht, in0=x_tiles[tt],
                                     scalar1=rsig[:, tt, :], scalar2=mrs[:, tt, :],
                                     op0=OP.mult, op1=OP.subtract)
                h_tiles.append(ht)

            hT = sb_hT.tile([128, DC, 512], bf16, tag="hT")
            for dc in range(DC):
                pt = ps_tr.tile([128, 512], bf16, tag="ptr")
                for tt in range(4):
                    nc.tensor.transpose(pt[:, tt * 128:(tt + 1) * 128],
                                        h_tiles[tt][:, dc * 128:(dc + 1) * 128], ident)
                nc.any.tensor_copy(out=hT[:, dc, :], in_=pt)

            psq = ps_st.tile([128, 2, 512], f32, tag="pst")
            for dc in range(DC):
                nc.tensor.matmul(psq[:, 0, :], lhsT=wqk_sb[:, dc, 0:128],
                                 rhs=hT[:, dc, :],
                                 start=(dc == 0), stop=(dc == DC - 1))
            for dc in range(DC):
                nc.tensor.matmul(psq[:, 1, :], lhsT=wqk_sb[:, dc, 128:256],
                                 rhs=hT[:, dc, :],
                                 start=(dc == 0), stop=(dc == DC - 1))
            rope_store(psq[:, 0, :], QT[:, b, l0:l0 + 512], l0)
            rope_store(psq[:, 1, :], KTb[:, b, l0:l0 + 512], l0)

            psv = ps_a.tile([128, 4, 128], f32, tag="psa")
            for tt in range(4):
                for dc in range(DC):
                    nc.tensor.matmul(psv[:, tt, :],
                                     lhsT=hT[:, dc, tt * 128:(tt + 1) * 128],
                                     rhs=wv_sb[:, dc, :],
                                     start=(dc == 0), stop=(dc == DC - 1))
            for tt in range(4):
                kt = cq * 4 + tt
                nc.any.tensor_copy(out=VT[:, b, kt, 0:64], in_=psv[:, tt, 0:64])
                nc.any.tensor_copy(out=VT[:, b, kt, 65:129], in_=psv[:, tt, 64:128])

            # software-pipeline with a two-chunk lag: block g-2's K/V prefix
            # completed two chunks ago (a full chunk of slack), attention
            # starts at chunk 2, and only two blocks remain as the tail
            if g >= 2:
                attention_block(*divmod(g - 2, CPB))
        for g in (G - 2, G - 1):
            attention_block(*divmod(g, CPB))

    nc.compile()
    return nc


def _perm_deinterleave():
    return np.concatenate([np.arange(0, 32, 2), np.arange(1, 32, 2),
                           np.arange(32, 64, 2), np.arange(33, 64, 2)])


def make_core_inputs(x, W_in, W_o, core, L):
    ha, hb = HPC * core, HPC * core + 1
    perm = _perm_deinterleave()

    def qk_rows(base, h):
        rows = W_in[base + h * D_K: base + (h + 1) * D_K, :]
        return rows[perm, :]

    wqkT = np.concatenate([
        qk_rows(0, ha), qk_rows(0, hb),
        qk_rows(D_MODEL, ha), qk_rows(D_MODEL, hb),
    ], axis=0).T.astype(BF)
    wvT = np.concatenate([
        W_in[2 * D_MODEL + ha * D_K: 2 * D_MODEL + (ha + 1) * D_K, :],
        W_in[2 * D_MODEL + hb * D_K: 2 * D_MODEL + (hb + 1) * D_K, :],
    ], axis=0).T.astype(BF)
    cols = np.concatenate([np.arange(ha * D_K, (ha + 1) * D_K),
                           np.arange(hb * D_K, (hb + 1) * D_K)])
    woT = W_o[:, cols].T.astype(BF)

    inv_freq = 1.0 / (ROPE_BASE ** (np.arange(32, dtype=np.float64) * 2.0 / D_K))
    ang = np.arange(L, dtype=np.float64)[:, None] * inv_freq[None, :]
    c32 = np.cos(ang).T.astype(np.float32)
    s32 = np.sin(ang).T.astype(np.float32)
    cosQ = [np.concatenate([c32[p:p + 16], c32[p:p + 16]]) for p in (0, 16)]
    sinQ = [np.concatenate([-s32[p:p + 16], s32[p:p + 16]]) for p in (0, 16)]
    cosT = np.concatenate(cosQ * 2, axis=0).astype(BF)
    sinsT = np.concatenate(sinQ * 2, axis=0).astype(BF)

    return {"x": np.ascontiguousarray(x.astype(BF)),
            "wqkT": np.ascontiguousarray(wqkT), "wvT": np.ascontiguousarray(wvT),
            "woT": np.ascontiguousarray(woT),
            "cosT": np.ascontiguousarray(cosT), "sinsT": np.ascontiguousarray(sinsT)}


_NC_CACHE = {}


def kernel(x, W_in, W_o):
    from concourse.bass_utils import run_bass_kernel_spmd
    x = np.asarray(x, dtype=np.float32)
    W_in = np.asarray(W_in, dtype=np.float32)
    W_o = np.asarray(W_o, dtype=np.float32)
    B, L, _ = x.shape
    key = (B, L)
    if key not in _NC_CACHE:
        _NC_CACHE[key] = build_nc(B, L)
    nc = _NC_CACHE[key]
    in_maps = [make_core_inputs(x, W_in, W_o, c, L) for c in range(N_CORES)]
    res = run_bass_kernel_spmd(nc, in_maps, core_ids=list(range(N_CORES)))
    out = np.zeros((B, L, D_MODEL), dtype=np.float32)
    for c in range(N_CORES):
        out += np.asarray(res.results[c]["out"]).astype(np.float32)
    return out



# revision 2
# speedup vs baseline: 3.8767x; 3.8767x over previous
"""Trainium2 Bass kernel for a causal multi-head attention block.

Reference computation (B=4, L=2048, D=1024, H=16, dk=64):
    h = LayerNorm(x); qkv = h @ W_in.T; q,k = rope(q),rope(k)
    o = causal_softmax(q k^T / 8) v;  out = o @ W_o.T

Sharding: Megatron-style tensor parallel over heads across 8 cores
(2 heads/core). x replicated (cast to bf16 host-side); W_in
column-sharded; W_o row-sharded; host sums the 8 bf16 partial outputs
in f32.

Device kernel layout: scores are computed transposed (S^T[k, q], keys
on partitions) with exp fused off the PSUM scores; A@V accumulates per
(q-tile, k-tile) with a ones-column in V producing softmax row-sums in
the same PSUM tile; o is normalized with a per-partition reciprocal,
transposed once, and fed to the W_o projection. x tiles are DMA'd one
chunk ahead, and the rope (x1,x2) pair swap is one DVE stream_shuffle
(rows are host-permuted so the swap is a within-quadrant 16-block
exchange). Attention for batch b-1 is software-pipelined between
batch b's LN/QKV chunks.
"""
import numpy as np
import ml_dtypes

import concourse.bass as bass
import concourse.bacc as bacc
import concourse.tile as tile
from concourse import mybir
from concourse.masks import make_identity

f32 = mybir.dt.float32
bf16 = mybir.dt.bfloat16
BF = ml_dtypes.bfloat16
AF = mybir.ActivationFunctionType
OP = mybir.AluOpType

D_MODEL = 1024
HEADS = 16
D_K = 64
N_CORES = 8
HPC = HEADS // N_CORES
ROPE_BASE = 10000.0
EPS = 1e-8
DC = D_MODEL // 128


def build_nc(B, L, variant="", reps=1):
    nc = bacc.Bacc("TRN2", target_bir_lowering=False)
    G = B * L // 512
    CPB = L // 512
    KT = L // 128
    QC = L // 512

    x_d = nc.dram_tensor("x", [B, L, D_MODEL], bf16, kind="ExternalInput")
    wqk_d = nc.dram_tensor("wqkT", [D_MODEL, 256], bf16, kind="ExternalInput")
    wv_d = nc.dram_tensor("wvT", [D_MODEL, 128], bf16, kind="ExternalInput")
    wo_d = nc.dram_tensor("woT", [128, D_MODEL], bf16, kind="ExternalInput")
    cos_d = nc.dram_tensor("cosT", [128, L], bf16, kind="ExternalInput")
    sins_d = nc.dram_tensor("sinsT", [128, L], bf16, kind="ExternalInput")
    out_d = nc.dram_tensor("out", [B, L, D_MODEL], bf16, kind="ExternalOutput")

    from contextlib import ExitStack
    with tile.TileContext(nc) as tc, ExitStack() as ctx:
        if reps > 1:
            ctx.enter_context(tc.For_i(0, reps, 1))
        csts = ctx.enter_context(tc.tile_pool(name="csts", bufs=1))
        sb_x = ctx.enter_context(tc.tile_pool(name="sb_x", bufs=8))
        sb_h = ctx.enter_context(tc.tile_pool(name="sb_h", bufs=6))
        sb_hT = ctx.enter_context(tc.tile_pool(name="sb_hT", bufs=2))
        sb_st = ctx.enter_context(tc.tile_pool(name="sb_st", bufs=6))
        sb_m = ctx.enter_context(tc.tile_pool(name="sb_m", bufs=4))
        sb_at = ctx.enter_context(tc.tile_pool(name="sb_at", bufs=3))
        sb_o = ctx.enter_context(tc.tile_pool(name="sb_o", bufs=3))
        sb_out = ctx.enter_context(tc.tile_pool(name="sb_out", bufs=4))
        ps_a = ctx.enter_context(tc.tile_pool(name="ps_a", bufs=2, space="PSUM"))
        ps_st = ctx.enter_context(tc.tile_pool(name="ps_st", bufs=2, space="PSUM"))
        ps_tr = ctx.enter_context(tc.tile_pool(name="ps_tr", bufs=2, space="PSUM"))

        # fetch chunk 0's x before the constant loads so LN stats can start
        # during the weight/table DMAs
        x_first = []
        for tt in range(4):
            xt = sb_x.tile([128, D_MODEL], bf16, tag="x", name=f"x0_{tt}")
            nc.sync.dma_start(out=xt, in_=x_d[0, tt * 128:(tt + 1) * 128, :])
            x_first.append(xt)

        wqk_sb = csts.tile([128, DC, 256], bf16)
        nc.sync.dma_start(out=wqk_sb, in_=wqk_d.rearrange("(c p) n -> p c n", p=128))
        wv_sb = csts.tile([128, DC, 128], bf16)
        nc.sync.dma_start(out=wv_sb, in_=wv_d.rearrange("(c p) n -> p c n", p=128))
        wo_sb = csts.tile([128, D_MODEL], bf16)
        nc.sync.dma_start(out=wo_sb, in_=wo_d[:, :])
        cos_sb = csts.tile([128, L], bf16)
        nc.sync.dma_start(out=cos_sb, in_=cos_d[:, :])
        sins_sb = csts.tile([128, L], bf16)
        nc.sync.dma_start(out=sins_sb, in_=sins_d[:, :])
        ident = csts.tile([128, 128], bf16)
        make_identity(nc, ident)
        mask = csts.tile([128, 128], bf16)
        nc.gpsimd.memset(mask, 1.0)
        nc.gpsimd.affine_select(out=mask, in_=mask, compare_op=OP.is_ge,
                                fill=0.0, base=0, pattern=[[1, 128]],
                                channel_multiplier=-1)

        QT = csts.tile([128, B, L], bf16)
        KTb = csts.tile([128, B, L], bf16)
        VT = csts.tile([128, B, KT, 130], bf16)
        nc.gpsimd.memset(VT[:, :, :, 64:65], 1.0)
        nc.gpsimd.memset(VT[:, :, :, 129:130], 1.0)

        SWAP16 = list(range(16, 32)) + list(range(16))

        def rope_store(src_ps, dst_ap, l0):
            s = sb_m.tile([128, 512], bf16, tag="qs")
            nc.any.tensor_copy(out=s, in_=src_ps)
            m1 = sb_m.tile([128, 512], bf16, tag="m1")
            nc.vector.tensor_tensor(out=m1, in0=s, in1=cos_sb[:, l0:l0 + 512],
                                    op=OP.mult)
            ssw = sb_m.tile([128, 512], bf16, tag="ssw")
            nc.vector.stream_shuffle(ssw, s, SWAP16)
            m2 = sb_m.tile([128, 512], bf16, tag="m2")
            nc.vector.tensor_tensor(out=m2, in0=ssw,
                                    in1=sins_sb[:, l0:l0 + 512], op=OP.mult)
            nc.vector.tensor_tensor(out=dst_ap, in0=m1, in1=m2, op=OP.add)

        def attention_block(b, qc):
            po_T = ps_tr.tile([128, 512], bf16, tag="ptr")
            nkt = 4 * qc + 4
            # Scores for BOTH heads interleaved: head a sits on PE array rows
            # 0-63 and head b on rows 64-127 (tile_position is inferred from
            # the lhsT base partition), so adjacent matmuls overlap in the
            # array. Diagonal k-tiles compute/exp only their valid query
            # suffix -- the per-qt AV loop reads At[:, 4qc+j, qt*128:] only
            # for qt >= j, so the invalid prefix is never consumed.
            Ats = [sb_at.tile([128, KT, 512], bf16, tag="at", name=f"at{h}")
                   for h in range(HPC)]
            for g2 in range(0, nkt, 2):
                psts = [ps_st.tile([128, 2, 512], f32, tag="pst", name=f"pst{h}")
                        for h in range(HPC)]
                for i in range(2):
                    kt = g2 + i
                    c0 = max(kt - 4 * qc, 0) * 128
                    for hh in range(HPC):
                        r0 = hh * 64
                        nc.tensor.matmul(
                            psts[hh][:, i, 0:512 - c0],
                            lhsT=KTb[r0:r0 + 64, b, kt * 128:(kt + 1) * 128],
                            rhs=QT[r0:r0 + 64, b, qc * 512 + c0:(qc + 1) * 512],
                            start=True, stop=True)
                if g2 + 1 < 4 * qc:        # both tiles whole: exp the pair
                    for hh in range(HPC):
                        nc.scalar.activation(Ats[hh][:, g2:g2 + 2, :], psts[hh],
                                             AF.Exp, scale=0.125)
                else:                      # diagonal tiles: exp valid suffix
                    for i in range(2):
                        kt = g2 + i
                        c0 = max(kt - 4 * qc, 0) * 128
                        for hh in range(HPC):
                            nc.scalar.activation(Ats[hh][:, kt, c0:512],
                                                 psts[hh][:, i, 0:512 - c0],
                                                 AF.Exp, scale=0.125)
            mask_eng = nc.vector if "mdve" in variant else nc.gpsimd
            for j in range(4):
                for hh in range(HPC):
                    blk = Ats[hh][:, 4 * qc + j, j * 128:(j + 1) * 128]
                    mask_eng.tensor_tensor(out=blk, in0=blk, in1=mask,
                                           op=OP.mult)
            for hh in range(HPC):
                r0 = hh * 64
                At = Ats[hh]
                po = ps_a.tile([128, 4, 65], f32, tag="psa")
                for qt in range(4):
                    lkt = 4 * qc + qt
                    for kt in range(lkt + 1):
                        nc.tensor.matmul(
                            po[:, qt, :],
                            lhsT=At[:, kt, qt * 128:(qt + 1) * 128],
                            rhs=VT[:, b, kt, hh * 65:hh * 65 + 65],
                            start=(kt == 0), stop=(kt == lkt))
                rec = sb_st.tile([128, 4, 1], f32, tag="rec")
                nc.vector.reciprocal(out=rec, in_=po[:, :, 64:65])
                o_sb = sb_o.tile([128, 4, 64], bf16, tag="osb")
                for qt in range(4):
                    nc.any.tensor_scalar(out=o_sb[:, qt, :],
                                         in0=po[:, qt, 0:64],
                                         scalar1=rec[:, qt, :], scalar2=None,
                                         op0=OP.mult)
                for qt in range(4):
                    nc.tensor.transpose(po_T[r0:r0 + 64, qt * 128:(qt + 1) * 128],
                                        o_sb[:, qt, :], ident)
            oT = sb_o.tile([128, 4, 128], bf16, tag="oT")
            nc.any.tensor_copy(out=oT, in_=po_T)
            for qt in range(4):
                pO1 = ps_a.tile([128, 512], f32, tag="psa")
                pO2 = ps_a.tile([128, 512], f32, tag="psa")
                nc.tensor.matmul(pO1, lhsT=oT[:, qt, :],
                                 rhs=wo_sb[:, 0:512], start=True, stop=True)
                nc.tensor.matmul(pO2, lhsT=oT[:, qt, :],
                                 rhs=wo_sb[:, 512:1024], start=True, stop=True)
                osb = sb_out.tile([128, D_MODEL], bf16, tag="outsb")
                nc.any.tensor_copy(out=osb[:, 0:512], in_=pO1)
                nc.any.tensor_copy(out=osb[:, 512:1024], in_=pO2)
                lq = qc * 512 + qt * 128
                nc.sync.dma_start(out=out_d[b, lq:lq + 128, :], in_=osb)

        def fetch_x(g):
            b, cq = divmod(g, CPB)
            l0 = cq * 512
            tiles = []
            for tt in range(4):
                xt = sb_x.tile([128, D_MODEL], bf16, tag="x")
                nc.sync.dma_start(out=xt, in_=x_d[b, l0 + tt * 128:l0 + (tt + 1) * 128, :])
                tiles.append(xt)
            return tiles

        x_pend = x_first
        for g in range(G):
            b, cq = divmod(g, CPB)
            l0 = cq * 512

            x_tiles = x_pend
            if g + 1 < G:
                x_pend = fetch_x(g + 1)
            mv = sb_st.tile([128, 4, 2], f32, tag="mv")
            for tt in range(4):
                st = sb_st.tile([128, 2, 6], f32, tag="stats")
                nc.vector.bn_stats(out=st[:, 0, :], in_=x_tiles[tt][:, 0:512])
                nc.vector.bn_stats(out=st[:, 1, :], in_=x_tiles[tt][:, 512:1024])
                nc.vector.bn_aggr(out=mv[:, tt, :], in_=st)

            i32 = mybir.dt.int32
            ve = sb_st.tile([128, 4, 1], f32, tag="ve")
            nc.vector.tensor_scalar(out=ve, in0=mv[:, :, 1:2], scalar1=EPS,
                                    scalar2=None, op0=OP.add)
            rsig = sb_st.tile([128, 4, 1], f32, tag="rsig")
            nc.vector.tensor_scalar(out=rsig.bitcast(i32), in0=ve.bitcast(i32),
                                    scalar1=1, scalar2=None,
                                    op0=OP.logical_shift_right)
            nc.vector.tensor_scalar(out=rsig.bitcast(i32), in0=rsig.bitcast(i32),
                                    scalar1=-1, scalar2=0x5f3759df,
                                    op0=OP.mult, op1=OP.add)
            nt = sb_st.tile([128, 4, 1], f32, tag="nt")
            for _ in range(2):
                nc.vector.tensor_tensor(out=nt, in0=rsig, in1=rsig, op=OP.mult)
                nc.vector.tensor_tensor(out=nt, in0=nt, in1=ve, op=OP.mult)
                nc.vector.tensor_scalar(out=nt, in0=nt, scalar1=-0.5, scalar2=1.5,
                                        op0=OP.mult, op1=OP.add)
                nc.vector.tensor_tensor(out=rsig, in0=rsig, in1=nt, op=OP.mult)
            mrs = sb_st.tile([128, 4, 1], f32, tag="mrs")
            nc.vector.tensor_tensor(out=mrs, in0=mv[:, :, 0:1], in1=rsig, op=OP.mult)

            h_tiles = []
            for tt in range(4):
                ht = sb_h.tile([128, D_MODEL], bf16, tag="h")
                nc.any.tensor_scalar(out=ht, in0=x_tiles[tt],
                                     scalar1=rsig[:, tt, :], scalar2=mrs[:, tt, :],
                                     op0=OP.mult, op1=OP.subtract)
                h_tiles.append(ht)

            hT = sb_hT.tile([128, DC, 512], bf16, tag="hT")
            for dc in range(DC):
                pt = ps_tr.tile([128, 512], bf16, tag="ptr")
                for tt in range(4):
                    nc.tensor.transpose(pt[:, tt * 128:(tt + 1) * 128],
                                        h_tiles[tt][:, dc * 128:(dc + 1) * 128], ident)
                nc.any.tensor_copy(out=hT[:, dc, :], in_=pt)

            psq = ps_st.tile([128, 2, 512], f32, tag="pst")
            for dc in range(DC):
                nc.tensor.matmul(psq[:, 0, :], lhsT=wqk_sb[:, dc, 0:128],
                                 rhs=hT[:, dc, :],
                                 start=(dc == 0), stop=(dc == DC - 1))
            for dc in range(DC):
                nc.tensor.matmul(psq[:, 1, :], lhsT=wqk_sb[:, dc, 128:256],
                                 rhs=hT[:, dc, :],
                                 start=(dc == 0), stop=(dc == DC - 1))
            rope_store(psq[:, 0, :], QT[:, b, l0:l0 + 512], l0)
            rope_store(psq[:, 1, :], KTb[:, b, l0:l0 + 512], l0)

            psv = ps_a.tile([128, 4, 128], f32, tag="psa")
            for tt in range(4):
                for dc in range(DC):
                    nc.tensor.matmul(psv[:, tt, :],
                                     lhsT=hT[:, dc, tt * 128:(tt + 1) * 128],
                                     rhs=wv_sb[:, dc, :],
                                     start=(dc == 0), stop=(dc == DC - 1))
            for tt in range(4):
                kt = cq * 4 + tt
                nc.any.tensor_copy(out=VT[:, b, kt, 0:64], in_=psv[:, tt, 0:64])
                nc.any.tensor_copy(out=VT[:, b, kt, 65:129], in_=psv[:, tt, 64:128])

            # software-pipeline with a two-chunk lag: block g-2's K/V prefix
            # completed two chunks ago (a full chunk of slack), attention
            # starts at chunk 2, and only two blocks remain as the tail
            if g >= 2:
                attention_block(*divmod(g - 2, CPB))
        for g in (G - 2, G - 1):
            attention_block(*divmod(g, CPB))

    nc.compile()
    return nc


def _perm_deinterleave():
    return np.concatenate([np.arange(0, 32, 2), np.arange(1, 32, 2),
                           np.arange(32, 64, 2), np.arange(33, 64, 2)])


def make_core_inputs(x, W_in, W_o, core, L):
    ha, hb = HPC * core, HPC * core + 1
    perm = _perm_deinterleave()

    def qk_rows(base, h):
        rows = W_in[base + h * D_K: base + (h + 1) * D_K, :]
        return rows[perm, :]

    wqkT = np.concatenate([
        qk_rows(0, ha), qk_rows(0, hb),
        qk_rows(D_MODEL, ha), qk_rows(D_MODEL, hb),
    ], axis=0).T.astype(BF)
    wvT = np.concatenate([
        W_in[2 * D_MODEL + ha * D_K: 2 * D_MODEL + (ha + 1) * D_K, :],
        W_in[2 * D_MODEL + hb * D_K: 2 * D_MODEL + (hb + 1) * D_K, :],
    ], axis=0).T.astype(BF)
    cols = np.concatenate([np.arange(ha * D_K, (ha + 1) * D_K),
                           np.arange(hb * D_K, (hb + 1) * D_K)])
    woT = W_o[:, cols].T.astype(BF)

    inv_freq = 1.0 / (ROPE_BASE ** (np.arange(32, dtype=np.float64) * 2.0 / D_K))
    ang = np.arange(L, dtype=np.float64)[:, None] * inv_freq[None, :]
    c32 = np.cos(ang).T.astype(np.float32)
    s32 = np.sin(ang).T.astype(np.float32)
    cosQ = [np.concatenate([c32[p:p + 16], c32[p:p + 16]]) for p in (0, 16)]
    sinQ = [np.concatenate([-s32[p:p + 16], s32[p:p + 16]]) for p in (0, 16)]
    cosT = np.concatenate(cosQ * 2, axis=0).astype(BF)
    sinsT = np.concatenate(sinQ * 2, axis=0).astype(BF)

    return {"x": np.ascontiguousarray(x.astype(BF)),
            "wqkT": np.ascontiguousarray(wqkT), "wvT": np.ascontiguousarray(wvT),
            "woT": np.ascontiguousarray(woT),
            "cosT": np.ascontiguousarray(cosT), "sinsT": np.ascontiguousarray(sinsT)}


_NC_CACHE = {}


def kernel(x, W_in, W_o):
    from concourse.bass_utils import run_bass_kernel_spmd
    x = np.asarray(x, dtype=np.float32)
    W_in = np.asarray(W_in, dtype=np.float32)
    W_o = np.asarray(W_o, dtype=np.float32)
    B, L, _ = x.shape
    key = (B, L)
    if key not in _NC_CACHE:
        _NC_CACHE[key] = build_nc(B, L)
    nc = _NC_CACHE[key]
    in_maps = [make_core_inputs(x, W_in, W_o, c, L) for c in range(N_CORES)]
    res = run_bass_kernel_spmd(nc, in_maps, core_ids=list(range(N_CORES)))
    out = np.zeros((B, L, D_MODEL), dtype=np.float32)
    for c in range(N_CORES):
        out += np.asarray(res.results[c]["out"]).astype(np.float32)
    return out

